# revision 9
# baseline (speedup 1.0000x reference)
"""Fused Linear + LayerNorm + residual-multiply kernel for 8 Trainium2 cores.

Computes, for full inputs x[B,1024], y[B,1024], weight[1024,1024], bias, gamma, beta:
    z  = x @ weight.T + bias
    ln = (z - mean(z)) * rsqrt(var(z) + eps) * gamma + beta     (over last dim)
    out = (ln + y) * y

Data-parallel over the batch dim: each of the 8 NeuronCores processes B/8 rows;
weight/bias/gamma/beta are replicated. No cross-core communication.

Per-core algorithm (b_core = B/8 rows, P=128, D=1024):
  - W.T (pre-transposed on host, [i,o] layout) resident in SBUF.
  - For each 128-row tile of x:
      * load x/y tiles (contiguous DMA)
      * transpose x tile 128x128 blocks on TensorE (via identity matmul) so the
        contraction dim i lands on partitions
      * 8 accumulating float32r matmuls per 512-wide output block (stationary =
        x.T block, moving = W.T block), plus a K=1 ones x bias matmul that adds
        the bias row inside PSUM
      * bn_stats/bn_aggr on VectorE for mean/var, sqrt(var+eps) on ScalarE,
        reciprocal on VectorE
      * ScalarE Identity-activation applies (z - mean) * rstd while copying
        PSUM -> SBUF (per-partition scale/bias operands)
      * VectorE tensor ops for (+ y) * y, then DMA the tile out
"""

import numpy as np
from contextlib import ExitStack

import concourse.bass as bass
import concourse.mybir as mybir
import concourse.tile as tile
from concourse import bacc, bass_utils
from concourse.masks import make_identity

P = 128
D = 1024
KT = D // P          # 8 k-tiles over the contraction dim
OB = 512             # o-block width (one PSUM bank of fp32)
N_CORES = 8
EPS = 1e-5

F32 = mybir.dt.float32
F32R = mybir.dt.float32r

AF = mybir.ActivationFunctionType
OP = mybir.AluOpType

_BUILD_CACHE = {}


def _build(b_core: int, trivial_affine: bool, mm_dtype=F32R):
    key = (b_core, trivial_affine, mm_dtype)
    if key in _BUILD_CACHE:
        return _BUILD_CACHE[key]

    nb = b_core // P
    nc = bacc.Bacc("TRN2", debug=False, num_devices=N_CORES)

    x = nc.dram_tensor("x", [b_core, D], F32, kind="ExternalInput").ap()
    y = nc.dram_tensor("y", [b_core, D], F32, kind="ExternalInput").ap()
    wt = nc.dram_tensor("wt", [D, D], F32, kind="ExternalInput").ap()  # W.T, [i, o]
    bias = nc.dram_tensor("bias", [D], F32, kind="ExternalInput").ap()
    if not trivial_affine:
        gamma = nc.dram_tensor("gamma", [D], F32, kind="ExternalInput").ap()
        beta = nc.dram_tensor("beta", [D], F32, kind="ExternalInput").ap()
    out = nc.dram_tensor("out", [b_core, D], F32, kind="ExternalOutput").ap()

    # Tiles feeding the matmuls are declared as mm_dtype (float32r) directly —
    # the BIR verifier requires every producer of an FP32r-matmul operand to
    # write FP32r output. DRAM-side APs are bitcast to match (same bit layout).
    def mmc(ap):
        return ap.bitcast(mm_dtype) if ap.dtype != mm_dtype else ap

    with tile.TileContext(nc) as tc, ExitStack() as ctx:
        const = ctx.enter_context(tc.tile_pool(name="const", bufs=1))
        xpool = ctx.enter_context(tc.tile_pool(name="xp", bufs=3))
        ypool = ctx.enter_context(tc.tile_pool(name="yp", bufs=3))
        xtp = ctx.enter_context(tc.tile_pool(name="xtp", bufs=2))
        tpool = ctx.enter_context(tc.tile_pool(name="tp", bufs=2))
        opool = ctx.enter_context(tc.tile_pool(name="op", bufs=2))
        stat = ctx.enter_context(tc.tile_pool(name="stat", bufs=4))
        psz = ctx.enter_context(tc.tile_pool(name="psz", bufs=2, space="PSUM"))
        pst = ctx.enter_context(tc.tile_pool(name="pst", bufs=2, space="PSUM"))

        # --- constants ---
        wt_sb = const.tile([P, KT, D], mm_dtype)  # [i_local, k, o]
        nc.sync.dma_start(
            out=wt_sb[:], in_=mmc(wt.rearrange("(k p) o -> p k o", p=P))
        )
        bias_sb = const.tile([1, D], mm_dtype)
        nc.sync.dma_start(out=bias_sb[:], in_=mmc(bias.unsqueeze(0)))
        ones_f32 = const.tile([1, P], F32)
        nc.vector.memset(ones_f32[:], 1.0)
        ones_sb = const.tile([1, P], mm_dtype)
        nc.scalar.activation(ones_sb[:], ones_f32[:], AF.Copy)
        ident = const.tile([P, P], F32)
        make_identity(nc, ident[:])
        eps_sb = const.tile([P, 1], F32)
        nc.vector.memset(eps_sb[:], EPS)
        if not trivial_affine:
            gamma_sb = const.tile([P, D], F32)
            nc.sync.dma_start(out=gamma_sb[:], in_=gamma.unsqueeze(0).to_broadcast([P, D]))
            beta_sb = const.tile([P, D], F32)
            nc.sync.dma_start(out=beta_sb[:], in_=beta.unsqueeze(0).to_broadcast([P, D]))

        for bt in range(nb):
            rows = slice(bt * P, (bt + 1) * P)
            x_sb = xpool.tile([P, D], F32)
            nc.sync.dma_start(out=x_sb[:], in_=x[rows, :])
            y_sb = ypool.tile([P, D], F32)
            nc.sync.dma_start(out=y_sb[:], in_=y[rows, :])

            # --- transpose x tile: 8x [128,128] via TensorE, staged in PSUM ---
            xt_sb = xtp.tile([P, KT, P], mm_dtype)  # [i_local, k, b_local]
            for half in range(2):
                ps_t = pst.tile([P, 4, P], F32)
                for j in range(4):
                    k = half * 4 + j
                    nc.tensor.transpose(
                        ps_t[:, j, :], x_sb[:, bass.ts(k, P)], ident[:]
                    )
                nc.scalar.activation(
                    xt_sb[:, bass.ts(half, 4), :], ps_t[:], AF.Copy
                )

            # --- matmuls: z = x @ W.T + bias, accumulated in PSUM ---
            z_ps = psz.tile([P, D], F32)
            for k in range(KT):
                lhsT = xt_sb[:, k, :]
                for half in range(2):
                    nc.tensor.matmul(
                        z_ps[:, bass.ts(half, OB)],
                        lhsT,
                        wt_sb[:, k, bass.ts(half, OB)],
                        start=(k == 0),
                        stop=False,
                    )
            for half in range(2):
                nc.tensor.matmul(
                    z_ps[:, bass.ts(half, OB)],
                    ones_sb[:],
                    bias_sb[:, bass.ts(half, OB)],
                    start=False,
                    stop=True,
                )

            # --- layernorm stats ---
            st = stat.tile([P, 2, 6], F32)
            nc.vector.bn_stats(out=st[:, 0, :], in_=z_ps[:, 0:OB])
            nc.vector.bn_stats(out=st[:, 1, :], in_=z_ps[:, OB:D])
            mv = stat.tile([P, 2], F32)
            nc.vector.bn_aggr(out=mv[:], in_=st[:])
            std = stat.tile([P, 1], F32)
            nc.scalar.activation(std[:], mv[:, 1:2], AF.Sqrt, bias=eps_sb[:], scale=1.0)
            rstd = stat.tile([P, 1], F32)
            nc.vector.reciprocal(rstd[:], std[:])
            nmr = stat.tile([P, 1], F32)  # -mean * rstd
            nc.vector.scalar_tensor_tensor(
                out=nmr[:], in0=mv[:, 0:1], scalar=-1.0, in1=rstd[:],
                op0=OP.mult, op1=OP.mult,
            )

            # --- normalize: t = (z - mean) * rstd, PSUM -> SBUF on ScalarE ---
            t_sb = tpool.tile([P, D], F32)
            nc.scalar.activation(
                t_sb[:], z_ps[:], AF.Identity, bias=nmr[:], scale=rstd[:]
            )
            if not trivial_affine:
                nc.vector.tensor_mul(out=t_sb[:], in0=t_sb[:], in1=gamma_sb[:])
                nc.vector.tensor_add(out=t_sb[:], in0=t_sb[:], in1=beta_sb[:])

            # --- out = (t + y) * y ---
            o_sb = opool.tile([P, D], F32)
            nc.vector.tensor_add(out=o_sb[:], in0=t_sb[:], in1=y_sb[:])
            nc.vector.tensor_mul(out=o_sb[:], in0=o_sb[:], in1=y_sb[:])
            nc.sync.dma_start(out=out[rows, :], in_=o_sb[:])

    nc.finalize()
    _BUILD_CACHE[key] = nc
    return nc


def _run(nc, in_maps, **kwargs):
    return bass_utils.run_bass_kernel_spmd(
        nc, in_maps, core_ids=list(range(N_CORES)), **kwargs
    )


def _prepare(x, y, weight, bias, gamma, beta):
    x = np.ascontiguousarray(x, dtype=np.float32)
    y = np.ascontiguousarray(y, dtype=np.float32)
    weight = np.asarray(weight, dtype=np.float32)
    bias = np.ascontiguousarray(bias, dtype=np.float32)
    gamma = np.asarray(gamma, dtype=np.float32)
    beta = np.asarray(beta, dtype=np.float32)

    B, IN = x.shape
    assert IN == D and weight.shape == (D, D) and y.shape == (B, D)
    assert B % (N_CORES * P) == 0
    b_core = B // N_CORES

    trivial = bool(np.all(gamma == 1.0)) and bool(np.all(beta == 0.0))
    nc = _build(b_core, trivial)

    wt = np.ascontiguousarray(weight.T)
    in_maps = []
    for c in range(N_CORES):
        m = {
            "x": x[c * b_core:(c + 1) * b_core],
            "y": y[c * b_core:(c + 1) * b_core],
            "wt": wt,
            "bias": bias,
        }
        if not trivial:
            m["gamma"] = gamma
            m["beta"] = beta
        in_maps.append(m)
    return nc, in_maps


def kernel(x, y, weight, bias, gamma, beta):
    nc, in_maps = _prepare(x, y, weight, bias, gamma, beta)
    res = _run(nc, in_maps)
    return np.concatenate([r["out"] for r in res.results], axis=0)
